# revision 5
# baseline (speedup 1.0000x reference)
"""DigitCaps dynamic-routing kernel for 8 Trainium2 NeuronCores.

Math (reference):
    u_hat[b,c,u,k] = sum_i W[c,u,k,i] * x[b,i,c]          (B=32, I=16, C=8192, U=32, K=16)
    b_ij = 0
    repeat 3x:
        c_ij  = softmax(b_ij, axis=c)
        s     = sum_c c_ij[c,u] * u_hat[b,c,u,k]
        v     = squash(s)    (norm over u, per (b,k))
        b_ij += mean_b <u_hat, v>
    return v

Strategy: shard C across the 8 cores (C_LOC = 1024 each).  u_hat (537 MB) is
never materialized; instead W is streamed from HBM once per routing iteration
(3 passes, 33.5 MB/core/pass) and each pass fuses the previous iteration's
agreement update a_{t-1} with the current weighted sum s_t:

  per 128-channel tile (pass t >= 1):
    VX_i[c,uk] = sum_b x[b,i,c] * (v_{t-1}[b,uk]/B)   (16 f32r matmuls, PE, row-tiled)
    a[c,u]     = sum_{i,k} VX_i[c,(u,k)] * W[c,(u,k,i)]   (DVE mul + reduce)
    b_state   += a ; wexp = exp(b_state)                  (softmax numerator)
    W         *= wexp[c,u]  (in-place, DVE)
    s_part    += sum_{c,i} xT_i[c,b] * (wexp*W)[c,(u,k)]  (16 f32r matmuls, PE)
    Z_part    += sum_c wexp[c,u]                          (ones-matmul, PE)
  then one 70 KB AllReduce of (s_part, Z_part), and the squash is computed
  redundantly on every core.  Softmax max-subtraction is skipped: b_ij stays
  within [-0.6, 0.6] for this problem so exp() cannot overflow.

Matmuls run in float32r (TF32-like, full PE rate); everything else is fp32.
"""

import contextlib

import numpy as np
import concourse.bass as bass
import concourse.bacc as bacc
import concourse.tile as tile
import concourse.mybir as mybir
from concourse.bass_utils import run_bass_kernel_spmd

B, I, C, U, K = 32, 16, 8192, 32, 16
UK = U * K
KI = K * I
N_CORES = 8
C_LOC = C // N_CORES
NT = C_LOC // 128
NUM_ITERS = 3

f32 = mybir.dt.float32
f32r = mybir.dt.float32r
MUL = mybir.AluOpType.mult
ADD = mybir.AluOpType.add
Exp = mybir.ActivationFunctionType.Exp

_CACHE = {}


def _body(nc, w_in, xn_in, xt_in, v_out):
    tc_pools = [
        ("wpool", dict(bufs=3)),
        ("xpool", dict(bufs=1)),
        ("spool", dict(bufs=1)),
        ("prodpool", dict(bufs=1)),
        ("small", dict(bufs=1)),
        ("pvx", dict(bufs=4, space="PSUM")),
        ("pacc", dict(bufs=1, space="PSUM")),
        ("dram", dict(bufs=1, space="DRAM")),
    ]
    with tile.TileContext(nc) as tc, contextlib.ExitStack() as stack:
        pools = [stack.enter_context(tc.tile_pool(name=n, **kw)) for n, kw in tc_pools]
        wpool, xpool, spool, prodpool, small, pvx, pacc, dram = pools

        # ---- persistent tiles ----
        # xn: [(i%4, b) = 128 partitions, (i//4, c) free]
        xn = xpool.tile([128, 4 * C_LOC], f32r)
        nc.sync.dma_start(xn[:], xn_in[:])
        xt = xpool.tile([128, NT * I * B], f32r)
        nc.sync.dma_start(xt[:], xt_in[:])
        ones_f = xpool.tile([128, B], f32)
        nc.vector.memset(ones_f[:], 1.0)
        b_state = spool.tile([128, NT * U], f32)
        nc.vector.memset(b_state[:], 0.0)
        vmat = spool.tile([128, UK], f32r)  # v_t/B replicated on 4 partition groups

        xn3 = xn[:].rearrange("p (il c) -> p il c", il=4)
        xt4 = xt[:].rearrange("c (n i b) -> c n i b", n=NT, i=I)

        for t in range(NUM_ITERS):
            ps_s = pacc.tile([B, UK], f32, tag="ps_s")
            ps_z = pacc.tile([B, U], f32, tag="ps_z")
            for n in range(NT):
                wt = wpool.tile([128, U * K * I], f32r, tag="w")
                nc.sync.dma_start(wt[:], w_in[bass.ts(n, 128), :])
                # per-i view of W: [c, i, u, k] (walk u stride 256, k stride 16)
                w4 = wt[:].rearrange("c (u k i) -> c i u k", u=U, k=K)
                if t > 0:
                    prod = prodpool.tile([128, U * I * K], f32, tag="prod")
                    # prod layout (u, i, k): reduce over (i,k) is contiguous per u
                    prod4 = prod[:].rearrange("c (u i k) -> c i u k", u=U, i=I)
                    for i in range(I):
                        ih, il = i % 4, i // 4
                        pv = pvx.tile([128, UK], f32, tag="pv")
                        nc.tensor.matmul(
                            pv[:],
                            xn3[32 * ih : 32 * (ih + 1), il, bass.ts(n, 128)],
                            vmat[32 * ih : 32 * (ih + 1), :],
                            start=True,
                            stop=True,
                            tile_position=(32 * ih, 0),
                        )
                        nc.vector.tensor_tensor(
                            out=prod4[:, i],
                            in0=pv[:].rearrange("c (u k) -> c u k", u=U),
                            in1=w4[:, i],
                            op=MUL,
                        )
                    a_red = small.tile([128, U], f32, tag="a_red")
                    nc.vector.tensor_reduce(
                        out=a_red[:],
                        in_=prod[:].rearrange("c (u r) -> c u r", u=U),
                        axis=mybir.AxisListType.X,
                        op=ADD,
                    )
                    b_slice = b_state[:, bass.ts(n, U)]
                    nc.vector.tensor_tensor(
                        out=b_slice, in0=b_slice, in1=a_red[:], op=ADD
                    )
                    wexp = small.tile([128, U], f32, tag="wexp")
                    nc.scalar.activation(wexp[:], b_slice, Exp)
                    nc.tensor.matmul(
                        ps_z[:],
                        ones_f[:],  # [128, 32] fp32 (tiny matmul, 4 cyc/row)
                        wexp[:],  # [128, 32]
                        start=(n == 0),
                        stop=(n == NT - 1),
                    )
                    # W *= wexp (in place) -> weighted W for the s-matmuls
                    w_u_r = wt[:].rearrange("c (u r) -> c u r", u=U)
                    nc.vector.tensor_tensor(
                        out=w_u_r,
                        in0=w_u_r,
                        in1=wexp[:].broadcast_to([128, U, KI]),
                        op=MUL,
                    )
                for i in range(I):
                    nc.tensor.matmul(
                        ps_s[:],
                        xt4[:, n, i, :],  # [128c, 32b]
                        w4[:, i],  # [128c, U, K]
                        start=(n == 0 and i == 0),
                        stop=(n == NT - 1 and i == I - 1),
                    )

            # ---- AllReduce of (s_part, Z_part) ----
            sz = small.tile([B, UK + U], f32, tag="sz")
            nc.vector.tensor_copy(out=sz[:, :UK], in_=ps_s[:])
            if t > 0:
                nc.vector.tensor_copy(out=sz[:, UK:], in_=ps_z[:])
            else:
                nc.vector.memset(sz[:, UK:], 0.0)
            cc_in = dram.tile([B, UK + U], f32, tag="cc_in")
            cc_out = dram.tile([B, UK + U], f32, tag="cc_out")
            nc.sync.dma_start(cc_in[:], sz[:])
            nc.gpsimd.collective_compute(
                "AllReduce",
                ADD,
                replica_groups=[list(range(N_CORES))],
                ins=[cc_in.opt()],
                outs=[cc_out.opt()],
            )
            # replicate the 32-row result to all 128 partitions (4 groups)
            sz_all = small.tile([128, UK + U], f32, tag="sz_all")
            for g in range(4):
                nc.sync.dma_start(sz_all[32 * g : 32 * (g + 1), :], cc_out[:])

            # ---- softmax-normalize s, squash into v (on all 128 partitions) ----
            s_n = small.tile([128, UK], f32, tag="s_n")
            if t == 0:
                nc.scalar.mul(s_n[:], sz_all[:, :UK], 1.0 / C)
            else:
                rz = small.tile([128, U], f32, tag="rz")
                nc.vector.reciprocal(rz[:], sz_all[:, UK:])
                nc.vector.tensor_tensor(
                    out=s_n[:].rearrange("b (u k) -> b u k", u=U),
                    in0=sz_all[:, :UK].rearrange("b (u k) -> b u k", u=U),
                    in1=rz[:].broadcast_to([128, U, K]),
                    op=MUL,
                )
            sq = small.tile([128, UK], f32, tag="sq")
            nc.scalar.square(sq[:], s_n[:])
            mag_sq = small.tile([128, K], f32, tag="mag_sq")
            nc.vector.tensor_reduce(
                out=mag_sq[:],
                in_=sq[:].rearrange("b (u k) -> b k u", u=U),
                axis=mybir.AxisListType.X,
                op=ADD,
            )
            mag = small.tile([128, K], f32, tag="mag")
            nc.scalar.sqrt(mag[:], mag_sq[:])
            den = small.tile([128, K], f32, tag="den")
            nc.vector.tensor_scalar_add(out=den[:], in0=mag_sq[:], scalar1=1.0)
            nc.vector.tensor_tensor(out=den[:], in0=den[:], in1=mag[:], op=MUL)
            rden = small.tile([128, K], f32, tag="rden")
            nc.vector.reciprocal(rden[:], den[:])
            fac = small.tile([128, K], f32, tag="fac")
            nc.vector.tensor_tensor(out=fac[:], in0=mag_sq[:], in1=rden[:], op=MUL)
            v_t = small.tile([128, UK], f32, tag="v_t")
            nc.vector.tensor_tensor(
                out=v_t[:].rearrange("b (u k) -> b k u", u=U),
                in0=s_n[:].rearrange("b (u k) -> b k u", u=U),
                in1=fac[:].broadcast_to([128, K, U]),
                op=MUL,
            )
            if t < NUM_ITERS - 1:
                nc.scalar.mul(vmat[:], v_t[:], 1.0 / B)
            else:
                nc.sync.dma_start(v_out[:], v_t[:B, :])


def _build():
    if "nc" in _CACHE:
        return _CACHE["nc"]
    nc = bacc.Bacc(
        "TRN2", target_bir_lowering=False, debug=False, num_devices=N_CORES
    )
    w_in = nc.dram_tensor("w", [C_LOC, U * K * I], f32r, kind="ExternalInput").ap()
    xn_in = nc.dram_tensor("xn", [128, 4 * C_LOC], f32r, kind="ExternalInput").ap()
    xt_in = nc.dram_tensor("xt", [128, NT * I * B], f32r, kind="ExternalInput").ap()
    v_out = nc.dram_tensor("v_out", [B, UK], f32, kind="ExternalOutput").ap()
    _body(nc, w_in, xn_in, xt_in, v_out)
    nc.compile()
    _CACHE["nc"] = nc
    return nc


def _prep_inputs(x, W):
    """Shard FULL inputs into the per-core DMA-friendly layouts."""
    x = np.asarray(x, dtype=np.float32)
    W = np.asarray(W, dtype=np.float32)
    in_maps = []
    for r in range(N_CORES):
        w_r = np.ascontiguousarray(W[r * C_LOC : (r + 1) * C_LOC]).reshape(C_LOC, -1)
        xs = x[:, :, r * C_LOC : (r + 1) * C_LOC]  # [B, I, C_LOC] view
        # xn[32*(i%4) + b, (i//4)*C_LOC + c] = xs[b, i, c]
        xn_r = np.ascontiguousarray(
            xs.transpose(1, 0, 2).reshape(4, 4, B, C_LOC).transpose(1, 2, 0, 3)
        ).reshape(128, 4 * C_LOC)
        # xt[cc, (tile, i, b)] = xs[b, i, tile*128 + cc]
        xt_r = np.ascontiguousarray(
            xs.reshape(B, I, NT, 128).transpose(3, 2, 1, 0)
        ).reshape(128, NT * I * B)
        in_maps.append({"w": w_r, "xn": xn_r, "xt": xt_r})
    return in_maps


def kernel(x, W):
    nc = _build()
    in_maps = _prep_inputs(x, W)
    res = run_bass_kernel_spmd(nc, in_maps, core_ids=list(range(N_CORES)))
    v = res.results[0]["v_out"]
    return v.reshape(B, U, K, 1).astype(np.float32)


# revision 6
# speedup vs baseline: 459.6176x; 459.6176x over previous
"""DigitCaps dynamic-routing kernel for 8 Trainium2 NeuronCores.

Math (reference):
    u_hat[b,c,u,k] = sum_i W[c,u,k,i] * x[b,i,c]          (B=32, I=16, C=8192, U=32, K=16)
    b_ij = 0
    repeat 3x:
        c_ij  = softmax(b_ij, axis=c)
        s     = sum_c c_ij[c,u] * u_hat[b,c,u,k]
        v     = squash(s)    (norm over u, per (b,k))
        b_ij += mean_b <u_hat, v>
    return v

Strategy: shard C across the 8 cores (C_LOC = 1024 each).  u_hat (537 MB) is
never materialized; instead W is streamed from HBM once per routing iteration
(3 passes, 33.5 MB/core/pass) and each pass fuses the previous iteration's
agreement update a_{t-1} with the current weighted sum s_t:

  per 128-channel tile (pass t >= 1):
    VX_i[c,uk] = sum_b x[b,i,c] * (v_{t-1}[b,uk]/B)   (16 f32r matmuls, PE, row-tiled)
    a[c,u]     = sum_{i,k} VX_i[c,(u,k)] * W[c,(u,k,i)]   (DVE mul + reduce)
    b_state   += a ; wexp = exp(b_state)                  (softmax numerator)
    W         *= wexp[c,u]  (in-place, DVE)
    s_part    += sum_{c,i} xT_i[c,b] * (wexp*W)[c,(u,k)]  (16 f32r matmuls, PE)
    Z_part    += sum_c wexp[c,u]                          (ones-matmul, PE)
  then one 70 KB AllReduce of (s_part, Z_part), and the squash is computed
  redundantly on every core.  Softmax max-subtraction is skipped: b_ij stays
  within [-0.6, 0.6] for this problem so exp() cannot overflow.

Matmuls run in float32r (TF32-like, full PE rate); everything else is fp32.
"""

import contextlib

import numpy as np
import concourse.bass as bass
import concourse.bacc as bacc
import concourse.tile as tile
import concourse.mybir as mybir
from concourse.bass_utils import run_bass_kernel_spmd

B, I, C, U, K = 32, 16, 8192, 32, 16
UK = U * K
KI = K * I
N_CORES = 8
C_LOC = C // N_CORES
NT = C_LOC // 128
NUM_ITERS = 3

f32 = mybir.dt.float32
f32r = mybir.dt.float32r
MUL = mybir.AluOpType.mult
ADD = mybir.AluOpType.add
Exp = mybir.ActivationFunctionType.Exp

_CACHE = {}


def _body(nc, w_in, xn_in, xt_in, v_out):
    tc_pools = [
        ("wpool", dict(bufs=3)),
        ("xpool", dict(bufs=1)),
        ("spool", dict(bufs=1)),
        ("prodpool", dict(bufs=1)),
        ("small", dict(bufs=1)),
        ("pvx", dict(bufs=4, space="PSUM")),
        ("pacc", dict(bufs=1, space="PSUM")),
        ("dram", dict(bufs=1, space="DRAM")),
    ]
    with tile.TileContext(nc) as tc, contextlib.ExitStack() as stack:
        pools = [stack.enter_context(tc.tile_pool(name=n, **kw)) for n, kw in tc_pools]
        wpool, xpool, spool, prodpool, small, pvx, pacc, dram = pools

        # ---- persistent tiles ----
        # xn: [(i%4, b) = 128 partitions, (i//4, c) free]
        xn = xpool.tile([128, 4 * C_LOC], f32r)
        nc.sync.dma_start(xn[:], xn_in[:])
        xt = xpool.tile([128, NT * I * B], f32r)
        nc.sync.dma_start(xt[:], xt_in[:])
        ones_f = xpool.tile([128, B], f32)
        nc.vector.memset(ones_f[:], 1.0)
        b_state = spool.tile([128, NT * U], f32)
        nc.vector.memset(b_state[:], 0.0)
        vmat = spool.tile([128, UK], f32r)  # v_t/B replicated on 4 partition groups

        xn3 = xn[:].rearrange("p (il c) -> p il c", il=4)
        xt4 = xt[:].rearrange("c (n i b) -> c n i b", n=NT, i=I)

        for t in range(NUM_ITERS):
            ps_s = pacc.tile([B, UK], f32, tag="ps_s")
            ps_z = pacc.tile([B, U], f32, tag="ps_z")
            for n in range(NT):
                wt = wpool.tile([128, U * K * I], f32r, tag="w")
                nc.sync.dma_start(wt[:], w_in[bass.ts(n, 128), :])
                # per-i view of W: [c, i, u, k] (walk u stride 256, k stride 16)
                w4 = wt[:].rearrange("c (u k i) -> c i u k", u=U, k=K)
                if t > 0:
                    prod = prodpool.tile([128, U * I * K], f32, tag="prod")
                    # prod layout (u, i, k): reduce over (i,k) is contiguous per u
                    prod4 = prod[:].rearrange("c (u i k) -> c i u k", u=U, i=I)
                    for i in range(I):
                        ih, il = i % 4, i // 4
                        pv = pvx.tile([128, UK], f32, tag="pv")
                        nc.tensor.matmul(
                            pv[:],
                            xn3[32 * ih : 32 * (ih + 1), il, bass.ts(n, 128)],
                            vmat[32 * ih : 32 * (ih + 1), :],
                            start=True,
                            stop=True,
                            tile_position=(32 * ih, 0),
                        )
                        nc.vector.tensor_tensor(
                            out=prod4[:, i],
                            in0=pv[:].rearrange("c (u k) -> c u k", u=U),
                            in1=w4[:, i],
                            op=MUL,
                        )
                    a_red = small.tile([128, U], f32, tag="a_red")
                    nc.vector.tensor_reduce(
                        out=a_red[:],
                        in_=prod[:].rearrange("c (u r) -> c u r", u=U),
                        axis=mybir.AxisListType.X,
                        op=ADD,
                    )
                    b_slice = b_state[:, bass.ts(n, U)]
                    nc.vector.tensor_tensor(
                        out=b_slice, in0=b_slice, in1=a_red[:], op=ADD
                    )
                    wexp = small.tile([128, U], f32, tag="wexp")
                    nc.scalar.activation(wexp[:], b_slice, Exp)
                    nc.tensor.matmul(
                        ps_z[:],
                        ones_f[:],  # [128, 32] fp32 (tiny matmul, 4 cyc/row)
                        wexp[:],  # [128, 32]
                        start=(n == 0),
                        stop=(n == NT - 1),
                    )
                    # W *= wexp (in place) -> weighted W for the s-matmuls
                    w_u_r = wt[:].rearrange("c (u r) -> c u r", u=U)
                    nc.vector.tensor_tensor(
                        out=w_u_r,
                        in0=w_u_r,
                        in1=wexp[:].broadcast_to([128, U, KI]),
                        op=MUL,
                    )
                for i in range(I):
                    nc.tensor.matmul(
                        ps_s[:],
                        xt4[:, n, i, :],  # [128c, 32b]
                        w4[:, i],  # [128c, U, K]
                        start=(n == 0 and i == 0),
                        stop=(n == NT - 1 and i == I - 1),
                    )

            # ---- AllReduce of (s_part, Z_part) ----
            sz = small.tile([B, UK + U], f32, tag="sz")
            nc.vector.tensor_copy(out=sz[:, :UK], in_=ps_s[:])
            if t > 0:
                nc.vector.tensor_copy(out=sz[:, UK:], in_=ps_z[:])
            else:
                nc.vector.memset(sz[:, UK:], 0.0)
            cc_in = dram.tile([B, UK + U], f32, tag="cc_in")
            cc_out = dram.tile([B, UK + U], f32, tag="cc_out")
            nc.sync.dma_start(cc_in[:], sz[:])
            nc.gpsimd.collective_compute(
                "AllReduce",
                ADD,
                replica_groups=[list(range(N_CORES))],
                ins=[cc_in.opt()],
                outs=[cc_out.opt()],
            )
            # replicate the 32-row result to all 128 partitions (4 groups)
            sz_all = small.tile([128, UK + U], f32, tag="sz_all")
            for g in range(4):
                nc.sync.dma_start(sz_all[32 * g : 32 * (g + 1), :], cc_out[:])

            # ---- softmax-normalize s, squash into v (on all 128 partitions) ----
            s_n = small.tile([128, UK], f32, tag="s_n")
            if t == 0:
                nc.scalar.mul(s_n[:], sz_all[:, :UK], 1.0 / C)
            else:
                rz = small.tile([128, U], f32, tag="rz")
                nc.vector.reciprocal(rz[:], sz_all[:, UK:])
                nc.vector.tensor_tensor(
                    out=s_n[:].rearrange("b (u k) -> b u k", u=U),
                    in0=sz_all[:, :UK].rearrange("b (u k) -> b u k", u=U),
                    in1=rz[:].broadcast_to([128, U, K]),
                    op=MUL,
                )
            sq = small.tile([128, UK], f32, tag="sq")
            nc.scalar.square(sq[:], s_n[:])
            mag_sq = small.tile([128, K], f32, tag="mag_sq")
            nc.vector.tensor_reduce(
                out=mag_sq[:],
                in_=sq[:].rearrange("b (u k) -> b k u", u=U),
                axis=mybir.AxisListType.X,
                op=ADD,
            )
            mag = small.tile([128, K], f32, tag="mag")
            nc.scalar.sqrt(mag[:], mag_sq[:])
            den = small.tile([128, K], f32, tag="den")
            nc.vector.tensor_scalar_add(out=den[:], in0=mag_sq[:], scalar1=1.0)
            nc.vector.tensor_tensor(out=den[:], in0=den[:], in1=mag[:], op=MUL)
            rden = small.tile([128, K], f32, tag="rden")
            nc.vector.reciprocal(rden[:], den[:])
            fac = small.tile([128, K], f32, tag="fac")
            nc.vector.tensor_tensor(out=fac[:], in0=mag_sq[:], in1=rden[:], op=MUL)
            v_t = small.tile([128, UK], f32, tag="v_t")
            nc.vector.tensor_tensor(
                out=v_t[:].rearrange("b (u k) -> b k u", u=U),
                in0=s_n[:].rearrange("b (u k) -> b k u", u=U),
                in1=fac[:].broadcast_to([128, K, U]),
                op=MUL,
            )
            if t < NUM_ITERS - 1:
                nc.scalar.mul(vmat[:], v_t[:], 1.0 / B)
            else:
                nc.sync.dma_start(v_out[:], v_t[:B, :])


def _build():
    if "nc" in _CACHE:
        return _CACHE["nc"]
    nc = bacc.Bacc(
        "TRN2", target_bir_lowering=False, debug=False, num_devices=N_CORES
    )
    w_in = nc.dram_tensor("w", [C_LOC, U * K * I], f32r, kind="ExternalInput").ap()
    xn_in = nc.dram_tensor("xn", [128, 4 * C_LOC], f32r, kind="ExternalInput").ap()
    xt_in = nc.dram_tensor("xt", [128, NT * I * B], f32r, kind="ExternalInput").ap()
    v_out = nc.dram_tensor("v_out", [B, UK], f32, kind="ExternalOutput").ap()
    _body(nc, w_in, xn_in, xt_in, v_out)
    nc.compile()
    _CACHE["nc"] = nc
    return nc


def _prep_inputs(x, W):
    """Shard FULL inputs into the per-core DMA-friendly layouts."""
    x = np.asarray(x, dtype=np.float32)
    W = np.asarray(W, dtype=np.float32)
    in_maps = []
    for r in range(N_CORES):
        w_r = np.ascontiguousarray(W[r * C_LOC : (r + 1) * C_LOC]).reshape(C_LOC, -1)
        xs = x[:, :, r * C_LOC : (r + 1) * C_LOC]  # [B, I, C_LOC] view
        # xn[32*(i%4) + b, (i//4)*C_LOC + c] = xs[b, i, c]
        xn_r = np.ascontiguousarray(
            xs.transpose(1, 0, 2).reshape(4, 4, B, C_LOC).transpose(1, 2, 0, 3)
        ).reshape(128, 4 * C_LOC)
        # xt[cc, (tile, i, b)] = xs[b, i, tile*128 + cc]
        xt_r = np.ascontiguousarray(
            xs.reshape(B, I, NT, 128).transpose(3, 2, 1, 0)
        ).reshape(128, NT * I * B)
        in_maps.append({"w": w_r, "xn": xn_r, "xt": xt_r})
    return in_maps


def kernel(x, W):
    nc = _build()
    in_maps = _prep_inputs(x, W)
    res = run_bass_kernel_spmd(nc, in_maps, core_ids=list(range(N_CORES)))
    v = res.results[0]["v_out"]
    return v.reshape(B, U, K, 1).astype(np.float32)


def make_runner(nc, in_maps):
    """Device-resident repeat runner (timing infrastructure for test.py).

    Mirrors bass2jax.run_bass_via_pjrt's multi-core branch but keeps the
    jitted callable and device-resident inputs so executions can be queued
    asynchronously and timed without per-call host transfers.
    """
    import jax
    from concourse import bass2jax
    from concourse.bass2jax import _bass_exec_p, install_neuronx_cc_hook
    from jax.experimental.shard_map import shard_map
    from jax.sharding import Mesh, PartitionSpec, NamedSharding

    install_neuronx_cc_hook()
    n_cores = len(in_maps)
    partition_name = nc.partition_id_tensor.name if nc.partition_id_tensor else None
    in_names, out_names, out_avals, zero_outs = [], [], [], []
    for alloc in nc.m.functions[0].allocations:
        if not isinstance(alloc, mybir.MemoryLocationSet):
            continue
        name = alloc.memorylocations[0].name
        if alloc.kind == "ExternalInput":
            if name != partition_name:
                in_names.append(name)
        elif alloc.kind == "ExternalOutput":
            out_names.append(name)
            shape = tuple(alloc.tensor_shape)
            dtype = mybir.dt.np(alloc.dtype)
            out_avals.append(jax.core.ShapedArray(shape, dtype))
            zero_outs.append(np.zeros(shape, dtype))
    n_params = len(in_names)
    n_outs = len(out_avals)
    all_in_names = list(in_names) + out_names
    if partition_name is not None:
        all_in_names.append(partition_name)

    def _body(*args):
        operands = list(args)
        if partition_name is not None:
            operands.append(bass2jax.partition_id_tensor())
        outs = _bass_exec_p.bind(
            *operands,
            out_avals=tuple(out_avals),
            in_names=tuple(all_in_names),
            out_names=tuple(out_names),
            lowering_input_output_aliases=(),
            sim_require_finite=True,
            sim_require_nnan=True,
            nc=nc,
        )
        return tuple(outs)

    devices = jax.devices()[:n_cores]
    mesh = Mesh(np.asarray(devices), ("core",))
    in_specs = (PartitionSpec("core"),) * (n_params + n_outs)
    out_specs = (PartitionSpec("core"),) * len(out_names)
    donate = tuple(range(n_params, n_params + n_outs))
    sharded = jax.jit(
        shard_map(
            _body, mesh=mesh, in_specs=in_specs, out_specs=out_specs, check_rep=False
        ),
        donate_argnums=donate,
        keep_unused=True,
    )
    sh = NamedSharding(mesh, PartitionSpec("core"))
    concat_in = [
        jax.device_put(
            np.concatenate([np.asarray(in_maps[c][nm]) for c in range(n_cores)], 0),
            sh,
        )
        for nm in in_names
    ]
    for a in concat_in:
        a.block_until_ready()

    def run(n_iter=1):
        outs = None
        for _ in range(n_iter):
            zeros = [
                np.zeros((n_cores * z.shape[0], *z.shape[1:]), z.dtype)
                for z in zero_outs
            ]
            outs = sharded(*concat_in, *zeros)
        for o in outs:
            o.block_until_ready()
        return outs

    return run


# revision 12
# speedup vs baseline: 919.0059x; 1.9995x over previous
"""DigitCaps dynamic-routing kernel for 8 Trainium2 NeuronCores.

Math (reference):
    u_hat[b,c,u,k] = sum_i W[c,u,k,i] * x[b,i,c]          (B=32, I=16, C=8192, U=32, K=16)
    b_ij = 0
    repeat 3x:
        c_ij  = softmax(b_ij, axis=c)
        s     = sum_c c_ij[c,u] * u_hat[b,c,u,k]
        v     = squash(s)    (norm over u, per (b,k))
        b_ij += mean_b <u_hat, v>
    return v

Strategy: shard C across the 8 cores (C_LOC = 1024 each).  u_hat (537 MB) is
never materialized; instead W is streamed from HBM once per routing iteration
(3 passes, 33.5 MB/core/pass) and each pass fuses the previous iteration's
agreement update a_{t-1} with the current weighted sum s_t:

  per 128-channel tile (pass t >= 1):
    VX_i[c,uk] = sum_b x[b,i,c] * (v_{t-1}[b,uk]/B)   (16 f32r matmuls, PE, row-tiled)
    a[c,u]     = sum_{i,k} VX_i[c,(u,k)] * W[c,(u,k,i)]   (DVE mul + reduce)
    b_state   += a ; wexp = exp(b_state)                  (softmax numerator)
    W         *= wexp[c,u]  (in-place, DVE)
    s_part    += sum_{c,i} xT_i[c,b] * (wexp*W)[c,(u,k)]  (16 f32r matmuls, PE)
    Z_part    += sum_c wexp[c,u]                          (ones-matmul, PE)
  then one 70 KB AllReduce of (s_part, Z_part), and the squash is computed
  redundantly on every core.  Softmax max-subtraction is skipped: b_ij stays
  within [-0.6, 0.6] for this problem so exp() cannot overflow.

Matmuls run in float32r (TF32-like, full PE rate); everything else is fp32.
"""

import contextlib

import numpy as np
import concourse.bass as bass
import concourse.bacc as bacc
import concourse.tile as tile
import concourse.mybir as mybir
from concourse.bass_utils import run_bass_kernel_spmd

B, I, C, U, K = 32, 16, 8192, 32, 16
UK = U * K
KI = K * I
N_CORES = 8
C_LOC = C // N_CORES
NT = C_LOC // 128
NUM_ITERS = 3

f32 = mybir.dt.float32
f32r = mybir.dt.float32r
MUL = mybir.AluOpType.mult
ADD = mybir.AluOpType.add
Exp = mybir.ActivationFunctionType.Exp

_CACHE = {}


def _body(nc, w_in, xn_in, xt_in, v_out, fake_cc=False):
    IG = 4  # i's per VX matmul group (pv2 spans IG PSUM banks)
    NG = I // IG
    tc_pools = [
        ("wpool", dict(bufs=3)),
        ("xpool", dict(bufs=1)),
        ("spool", dict(bufs=1)),
        ("prodpool", dict(bufs=1)),
        ("small", dict(bufs=1)),
        ("pvx", dict(bufs=1, space="PSUM")),
        ("pacc", dict(bufs=1, space="PSUM")),
        ("dram", dict(bufs=1, space="DRAM")),
    ]
    with tile.TileContext(nc) as tc, contextlib.ExitStack() as stack:
        pools = [stack.enter_context(tc.tile_pool(name=n, **kw)) for n, kw in tc_pools]
        wpool, xpool, spool, prodpool, small, pvx, pacc, dram = pools

        # ---- persistent tiles ----
        # xn: [(i%4, b) = 128 partitions, (i//4, c) free]
        xn = xpool.tile([128, 4 * C_LOC], f32r)
        nc.sync.dma_start(xn[:], xn_in[:])
        xt = xpool.tile([128, NT * I * B], f32r)
        nc.sync.dma_start(xt[:], xt_in[:])
        ones_f = xpool.tile([128, B], f32)
        nc.vector.memset(ones_f[:], 1.0)
        b_state = spool.tile([128, NT * U], f32)
        nc.vector.memset(b_state[:], 0.0)
        wexp_state = spool.tile([128, NT * U], f32)
        # vblk: block-diagonal moving operand for the VX matmuls
        # vblk[32*g + b, g*UK + z] = v[b, z] / B ; off-diagonal zero
        vblk = spool.tile([128, 4 * UK], f32r)
        nc.vector.memset(vblk[:].bitcast(f32), 0.0)

        xn3 = xn[:].rearrange("p (il c) -> p il c", il=4)
        xt4 = xt[:].rearrange("c (n i b) -> c n i b", n=NT, i=I)

        for t in range(NUM_ITERS):
            ps_s = pacc.tile([B, UK], f32, tag="ps_s")
            for n in range(NT):
                wt = wpool.tile([128, U * K * I], f32r, tag="w")
                nc.sync.dma_start(wt[:], w_in[bass.ts(n, 128), :])
                # per-i view of W: [c, i, u, k] (walk u stride 256, k stride 16)
                w4 = wt[:].rearrange("c (u k i) -> c i u k", u=U, k=K)
                if t > 0:
                    prod = prodpool.tile([128, U * I * K], f32, tag="prod")
                    # prod layout (u, i, k): reduce over (i,k) contiguous per u
                    prod4 = prod[:].rearrange("c (u i k) -> c i u k", u=U, i=I)
                    for il in range(NG):
                        pv2 = pvx.tile([128, IG * UK], f32, tag="pv2")
                        for ih in range(IG):
                            # i = 4*il + ih ; full-128 contraction, zero rows
                            # of vblk outside group ih contribute nothing
                            nc.tensor.matmul(
                                pv2[:, bass.ts(ih, UK)],
                                xn3[:, il, bass.ts(n, 128)],  # [128, 128]
                                vblk[:, bass.ts(ih, UK)],  # [128, 512]
                                start=True,
                                stop=True,
                            )
                        # prod[c, (u, 4il..4il+4, k)] = pv2 * W
                        nc.vector.tensor_tensor(
                            out=prod4[:, IG * il : IG * (il + 1)],
                            in0=pv2[:].rearrange("c (i z) -> c i z", i=IG).rearrange(
                                "c i (u k) -> c i u k", u=U
                            ),
                            in1=w4[:, IG * il : IG * (il + 1)],
                            op=MUL,
                        )
                    a_red = small.tile([128, U], f32, tag="a_red")
                    nc.vector.tensor_reduce(
                        out=a_red[:],
                        in_=prod[:].rearrange("c (u r) -> c u r", u=U),
                        axis=mybir.AxisListType.X,
                        op=ADD,
                    )
                    b_slice = b_state[:, bass.ts(n, U)]
                    nc.vector.tensor_tensor(
                        out=b_slice, in0=b_slice, in1=a_red[:], op=ADD
                    )
                    wexp = wexp_state[:, bass.ts(n, U)]
                    nc.scalar.activation(wexp, b_slice, Exp)
                    # W *= wexp (in place) -> weighted W for the s-matmuls
                    w_u_r = wt[:].rearrange("c (u r) -> c u r", u=U)
                    nc.vector.tensor_tensor(
                        out=w_u_r,
                        in0=w_u_r,
                        in1=wexp.broadcast_to([128, U, KI]),
                        op=MUL,
                    )
                for i in range(I):
                    nc.tensor.matmul(
                        ps_s[:],
                        xt4[:, n, i, :],  # [128c, 32b]
                        w4[:, i],  # [128c, U, K]
                        start=(n == 0 and i == 0),
                        stop=(n == NT - 1 and i == I - 1),
                    )

            # ---- Z = sum_c wexp (one matmul over the whole pass state) ----
            NZ = NT * U
            if t > 0:
                ps_z = pacc.tile([B, NZ], f32, tag="ps_z")
                nc.tensor.matmul(
                    ps_z[:], ones_f[:], wexp_state[:], start=True, stop=True
                )

            # ---- AllReduce of (s_part, Z_part) ----
            sz = small.tile([B, UK + NZ], f32, tag="sz")
            nc.vector.tensor_copy(out=sz[:, :UK], in_=ps_s[:])
            if t > 0:
                nc.vector.tensor_copy(out=sz[:, UK:], in_=ps_z[:])
            else:
                nc.vector.memset(sz[:, UK:], 0.0)
            cc_in = dram.tile([B, UK + NZ], f32, tag="cc_in")
            cc_out = dram.tile([B, UK + NZ], f32, tag="cc_out")
            nc.sync.dma_start(cc_in[:], sz[:])
            if fake_cc:
                nc.sync.dma_start(cc_out[:], cc_in[:])
            else:
                nc.gpsimd.collective_compute(
                    "AllReduce",
                    ADD,
                    replica_groups=[list(range(N_CORES))],
                    ins=[cc_in.opt()],
                    outs=[cc_out.opt()],
                )
            # replicate the 32-row result to all 128 partitions (4 groups)
            sz_all = small.tile([128, UK + NZ], f32, tag="sz_all")
            for g in range(4):
                nc.sync.dma_start(sz_all[32 * g : 32 * (g + 1), :], cc_out[:])

            # ---- softmax-normalize s, squash into v (on all 128 partitions) ----
            s_n = small.tile([128, UK], f32, tag="s_n")
            if t == 0:
                nc.scalar.mul(s_n[:], sz_all[:, :UK], 1.0 / C)
            else:
                zf = small.tile([128, U], f32, tag="zf")
                nc.vector.tensor_reduce(
                    out=zf[:],
                    in_=sz_all[:, UK:].rearrange("p (n u) -> p u n", n=NT),
                    axis=mybir.AxisListType.X,
                    op=ADD,
                )
                rz = small.tile([128, U], f32, tag="rz")
                nc.vector.reciprocal(rz[:], zf[:])
                nc.vector.tensor_tensor(
                    out=s_n[:].rearrange("b (u k) -> b u k", u=U),
                    in0=sz_all[:, :UK].rearrange("b (u k) -> b u k", u=U),
                    in1=rz[:].broadcast_to([128, U, K]),
                    op=MUL,
                )
            sq = small.tile([128, UK], f32, tag="sq")
            nc.scalar.square(sq[:], s_n[:])
            mag_sq = small.tile([128, K], f32, tag="mag_sq")
            nc.vector.tensor_reduce(
                out=mag_sq[:],
                in_=sq[:].rearrange("b (u k) -> b k u", u=U),
                axis=mybir.AxisListType.X,
                op=ADD,
            )
            mag = small.tile([128, K], f32, tag="mag")
            nc.scalar.sqrt(mag[:], mag_sq[:])
            den = small.tile([128, K], f32, tag="den")
            nc.vector.tensor_scalar_add(out=den[:], in0=mag_sq[:], scalar1=1.0)
            rden = small.tile([128, K], f32, tag="rden")
            nc.vector.reciprocal(rden[:], den[:])
            # fac = mag_sq / ((1 + mag_sq) * mag) = mag / (1 + mag_sq)
            fac = small.tile([128, K], f32, tag="fac")
            nc.vector.tensor_tensor(out=fac[:], in0=mag[:], in1=rden[:], op=MUL)
            if t < NUM_ITERS - 1:
                # write v/B into the 4 diagonal blocks of vblk
                facb = small.tile([128, K], f32, tag="facb")
                nc.vector.tensor_scalar_mul(out=facb[:], in0=fac[:], scalar1=1.0 / B)
                for g in range(4):
                    rows = slice(32 * g, 32 * (g + 1))
                    nc.vector.tensor_tensor(
                        out=vblk[rows, bass.ts(g, UK)].rearrange(
                            "b (u k) -> b k u", u=U
                        ),
                        in0=s_n[rows, :].rearrange("b (u k) -> b k u", u=U),
                        in1=facb[rows, :].broadcast_to([32, K, U]),
                        op=MUL,
                    )
            else:
                v_t = small.tile([B, UK], f32, tag="v_t")
                nc.vector.tensor_tensor(
                    out=v_t[:].rearrange("b (u k) -> b k u", u=U),
                    in0=s_n[:B, :].rearrange("b (u k) -> b k u", u=U),
                    in1=fac[:B, :].broadcast_to([B, K, U]),
                    op=MUL,
                )
                nc.sync.dma_start(v_out[:], v_t[:])


def _build():
    if "nc" in _CACHE:
        return _CACHE["nc"]
    nc = bacc.Bacc(
        "TRN2", target_bir_lowering=False, debug=False, num_devices=N_CORES
    )
    w_in = nc.dram_tensor("w", [C_LOC, U * K * I], f32r, kind="ExternalInput").ap()
    xn_in = nc.dram_tensor("xn", [128, 4 * C_LOC], f32r, kind="ExternalInput").ap()
    xt_in = nc.dram_tensor("xt", [128, NT * I * B], f32r, kind="ExternalInput").ap()
    v_out = nc.dram_tensor("v_out", [B, UK], f32, kind="ExternalOutput").ap()
    _body(nc, w_in, xn_in, xt_in, v_out)
    nc.compile()
    _CACHE["nc"] = nc
    return nc


def _prep_inputs(x, W):
    """Shard FULL inputs into the per-core DMA-friendly layouts."""
    x = np.asarray(x, dtype=np.float32)
    W = np.asarray(W, dtype=np.float32)
    in_maps = []
    for r in range(N_CORES):
        w_r = np.ascontiguousarray(W[r * C_LOC : (r + 1) * C_LOC]).reshape(C_LOC, -1)
        xs = x[:, :, r * C_LOC : (r + 1) * C_LOC]  # [B, I, C_LOC] view
        # xn[32*(i%4) + b, (i//4)*C_LOC + c] = xs[b, i, c]
        xn_r = np.ascontiguousarray(
            xs.transpose(1, 0, 2).reshape(4, 4, B, C_LOC).transpose(1, 2, 0, 3)
        ).reshape(128, 4 * C_LOC)
        # xt[cc, (tile, i, b)] = xs[b, i, tile*128 + cc]
        xt_r = np.ascontiguousarray(
            xs.reshape(B, I, NT, 128).transpose(3, 2, 1, 0)
        ).reshape(128, NT * I * B)
        in_maps.append({"w": w_r, "xn": xn_r, "xt": xt_r})
    return in_maps


def kernel(x, W):
    nc = _build()
    in_maps = _prep_inputs(x, W)
    res = run_bass_kernel_spmd(nc, in_maps, core_ids=list(range(N_CORES)))
    v = res.results[0]["v_out"]
    return v.reshape(B, U, K, 1).astype(np.float32)


def make_runner(nc, in_maps):
    """Device-resident repeat runner (timing infrastructure for test.py).

    Mirrors bass2jax.run_bass_via_pjrt's multi-core branch but keeps the
    jitted callable and device-resident inputs so executions can be queued
    asynchronously and timed without per-call host transfers.
    """
    import jax
    from concourse import bass2jax
    from concourse.bass2jax import _bass_exec_p, install_neuronx_cc_hook
    from jax.experimental.shard_map import shard_map
    from jax.sharding import Mesh, PartitionSpec, NamedSharding

    install_neuronx_cc_hook()
    n_cores = len(in_maps)
    partition_name = nc.partition_id_tensor.name if nc.partition_id_tensor else None
    in_names, out_names, out_avals, zero_outs = [], [], [], []
    for alloc in nc.m.functions[0].allocations:
        if not isinstance(alloc, mybir.MemoryLocationSet):
            continue
        name = alloc.memorylocations[0].name
        if alloc.kind == "ExternalInput":
            if name != partition_name:
                in_names.append(name)
        elif alloc.kind == "ExternalOutput":
            out_names.append(name)
            shape = tuple(alloc.tensor_shape)
            dtype = mybir.dt.np(alloc.dtype)
            out_avals.append(jax.core.ShapedArray(shape, dtype))
            zero_outs.append(np.zeros(shape, dtype))
    n_params = len(in_names)
    n_outs = len(out_avals)
    all_in_names = list(in_names) + out_names
    if partition_name is not None:
        all_in_names.append(partition_name)

    def _body(*args):
        operands = list(args)
        if partition_name is not None:
            operands.append(bass2jax.partition_id_tensor())
        outs = _bass_exec_p.bind(
            *operands,
            out_avals=tuple(out_avals),
            in_names=tuple(all_in_names),
            out_names=tuple(out_names),
            lowering_input_output_aliases=(),
            sim_require_finite=True,
            sim_require_nnan=True,
            nc=nc,
        )
        return tuple(outs)

    devices = jax.devices()[:n_cores]
    mesh = Mesh(np.asarray(devices), ("core",))
    in_specs = (PartitionSpec("core"),) * (n_params + n_outs)
    out_specs = (PartitionSpec("core"),) * len(out_names)
    donate = tuple(range(n_params, n_params + n_outs))
    sharded = jax.jit(
        shard_map(
            _body, mesh=mesh, in_specs=in_specs, out_specs=out_specs, check_rep=False
        ),
        donate_argnums=donate,
        keep_unused=True,
    )
    sh = NamedSharding(mesh, PartitionSpec("core"))
    concat_in = [
        jax.device_put(
            np.concatenate([np.asarray(in_maps[c][nm]) for c in range(n_cores)], 0),
            sh,
        )
        for nm in in_names
    ]
    for a in concat_in:
        a.block_until_ready()

    def run(n_iter=1):
        outs = None
        for _ in range(n_iter):
            zeros = [
                np.zeros((n_cores * z.shape[0], *z.shape[1:]), z.dtype)
                for z in zero_outs
            ]
            outs = sharded(*concat_in, *zeros)
        for o in outs:
            o.block_until_ready()
        return outs

    return run


# revision 17
# speedup vs baseline: 7651.1055x; 8.3254x over previous
"""DigitCaps dynamic-routing kernel for 8 Trainium2 NeuronCores.

Math (reference):
    u_hat[b,c,u,k] = sum_i W[c,u,k,i] * x[b,i,c]          (B=32, I=16, C=8192, U=32, K=16)
    b_ij = 0
    repeat 3x:
        c_ij  = softmax(b_ij, axis=c)
        s     = sum_c c_ij[c,u] * u_hat[b,c,u,k]
        v     = squash(s)    (norm over u, per (b,k))
        b_ij += mean_b <u_hat, v>
    return v

Strategy: shard C across the 8 cores (C_LOC = 1024 each).  u_hat (537 MB) is
never materialized; instead W is streamed from HBM once per routing iteration
(3 passes, 33.5 MB/core/pass) and each pass fuses the previous iteration's
agreement update a_{t-1} with the current weighted sum s_t:

  per 128-channel tile (pass t >= 1):
    VX_i[c,uk] = sum_b x[b,i,c] * (v_{t-1}[b,uk]/B)   (16 f32r matmuls via a
                 block-diagonal moving operand, grouped into double-buffered
                 multi-bank PSUM tiles so the DVE consumer pipelines)
    a[c,u]     = sum_{i,k} VX_i[c,(u,k)] * W[c,(u,k,i)]   (DVE mul + reduce)
    b_state   += a ; wexp = exp(b_state)                  (softmax numerator)
    W         *= wexp[c,u]  (in-place, DVE)
    s_part    += sum_{c,i} xT_i[c,b] * (wexp*W)[c,(u,k)]  (16 f32r matmuls, PE)
  Z = sum_c wexp is one ones-matmul over the stashed wexp state per pass.
  Then one ~100 KB AllReduce of (s_part, Z_part); the squash is computed
  redundantly on every core.  Softmax max-subtraction is skipped: b_ij stays
  within [-0.6, 0.6] for this problem so exp() cannot overflow.
  Dependent-instruction latency on this part (~2-4 us per edge) is the main
  non-DMA cost, so the structure favors few, large ops and deep pipelining.

Matmuls run in float32r (TF32-like, full PE rate); everything else is fp32.
"""

import contextlib

import numpy as np
import concourse.bass as bass
import concourse.bacc as bacc
import concourse.tile as tile
import concourse.mybir as mybir
from concourse.bass_utils import run_bass_kernel_spmd

B, I, C, U, K = 32, 16, 8192, 32, 16
UK = U * K
KI = K * I
N_CORES = 8
C_LOC = C // N_CORES
NT = C_LOC // 128
NUM_ITERS = 3

f32 = mybir.dt.float32
f32r = mybir.dt.float32r
MUL = mybir.AluOpType.mult
ADD = mybir.AluOpType.add
Exp = mybir.ActivationFunctionType.Exp

_CACHE = {}


def _body(nc, w_in, xn_in, xt_in, v_out, fake_cc=False, repeat=1, skip_a=False, skip_wd=False, skip_s=False, ig=2, pvbufs=2):
    IG = ig  # i's per VX matmul group (pv2 spans IG PSUM banks)
    NG = I // IG
    tc_pools = [
        ("wpool", dict(bufs=3)),
        ("xpool", dict(bufs=1)),
        ("spool", dict(bufs=1)),
        ("prodpool", dict(bufs=1)),
        ("small", dict(bufs=1)),
        ("pvx", dict(bufs=pvbufs, space="PSUM")),
        ("pacc", dict(bufs=1, space="PSUM")),
        ("dram", dict(bufs=1, space="DRAM")),
    ]
    with tile.TileContext(nc) as tc, contextlib.ExitStack() as stack:
        pools = [stack.enter_context(tc.tile_pool(name=n, **kw)) for n, kw in tc_pools]
        wpool, xpool, spool, prodpool, small, pvx, pacc, dram = pools

        # ---- persistent tiles ----
        # xn: [(i%4, b) = 128 partitions, (i//4, c) free]
        xn = xpool.tile([128, 4 * C_LOC], f32r)
        nc.sync.dma_start(xn[:], xn_in[:])
        xt = xpool.tile([128, NT * I * B], f32r)
        nc.sync.dma_start(xt[:], xt_in[:])
        ones_f = xpool.tile([128, B], f32)
        nc.vector.memset(ones_f[:], 1.0)
        b_state = spool.tile([128, NT * U], f32)
        nc.vector.memset(b_state[:], 0.0)
        wexp_state = spool.tile([128, NT * U], f32)
        # vblk: block-diagonal moving operand for the VX matmuls
        # vblk[32*g + b, g*UK + z] = v[b, z] / B ; off-diagonal zero
        vblk = spool.tile([128, 4 * UK], f32r)
        nc.vector.memset(vblk[:].bitcast(f32), 0.0)

        xn3 = xn[:].rearrange("p (il c) -> p il c", il=4)
        xt4 = xt[:].rearrange("c (n i b) -> c n i b", n=NT, i=I)

        for rep in range(repeat):
          if rep > 0:
            nc.vector.memset(b_state[:], 0.0)
          for t in range(NUM_ITERS):
            ps_s = pacc.tile([B, UK], f32, tag="ps_s")
            for n in range(NT):
                wt = wpool.tile([128, U * K * I], f32r, tag="w")
                nc.sync.dma_start(wt[:], w_in[bass.ts(n, 128), :])
                # per-i view of W: [c, i, u, k] (walk u stride 256, k stride 16)
                w4 = wt[:].rearrange("c (u k i) -> c i u k", u=U, k=K)
                if t > 0 and not skip_a:
                    prod = prodpool.tile([128, U * I * K], f32, tag="prod")
                    # prod layout (u, i, k): reduce over (i,k) contiguous per u
                    prod4 = prod[:].rearrange("c (u i k) -> c i u k", u=U, i=I)
                    for g in range(NG):
                        pv2 = pvx.tile([128, IG * UK], f32, tag="pv2")
                        for j in range(IG):
                            # i = g*IG + j ; full-128 contraction, zero rows
                            # of vblk outside group i%4 contribute nothing
                            i = g * IG + j
                            nc.tensor.matmul(
                                pv2[:, bass.ts(j, UK)],
                                xn3[:, i // 4, bass.ts(n, 128)],  # [128, 128]
                                vblk[:, bass.ts(i % 4, UK)],  # [128, 512]
                                start=True,
                                stop=True,
                            )
                        # prod[c, (u, g*IG..g*IG+IG, k)] = pv2 * W
                        nc.vector.tensor_tensor(
                            out=prod4[:, IG * g : IG * (g + 1)],
                            in0=pv2[:].rearrange("c (i z) -> c i z", i=IG).rearrange(
                                "c i (u k) -> c i u k", u=U
                            ),
                            in1=w4[:, IG * g : IG * (g + 1)],
                            op=MUL,
                        )
                    a_red = small.tile([128, U], f32, tag="a_red")
                    nc.vector.tensor_reduce(
                        out=a_red[:],
                        in_=prod[:].rearrange("c (u r) -> c u r", u=U),
                        axis=mybir.AxisListType.X,
                        op=ADD,
                    )
                    b_slice = b_state[:, bass.ts(n, U)]
                    nc.vector.tensor_tensor(
                        out=b_slice, in0=b_slice, in1=a_red[:], op=ADD
                    )
                    wexp = wexp_state[:, bass.ts(n, U)]
                    nc.scalar.activation(wexp, b_slice, Exp)
                    if not skip_wd:
                        # W *= wexp (in place) -> weighted W for the s-matmuls
                        w_u_r = wt[:].rearrange("c (u r) -> c u r", u=U)
                        nc.vector.tensor_tensor(
                            out=w_u_r,
                            in0=w_u_r,
                            in1=wexp.broadcast_to([128, U, KI]),
                            op=MUL,
                        )
                if t > 0 and skip_a:
                    b_slice = b_state[:, bass.ts(n, U)]
                    wexp = wexp_state[:, bass.ts(n, U)]
                    nc.scalar.activation(wexp, b_slice, Exp)
                if skip_s:
                    continue
                for i in range(I):
                    nc.tensor.matmul(
                        ps_s[:],
                        xt4[:, n, i, :],  # [128c, 32b]
                        w4[:, i],  # [128c, U, K]
                        start=(n == 0 and i == 0),
                        stop=(n == NT - 1 and i == I - 1),
                    )

            # ---- Z = sum_c wexp (one matmul over the whole pass state) ----
            NZ = NT * U
            if t > 0:
                ps_z = pacc.tile([B, NZ], f32, tag="ps_z")
                nc.tensor.matmul(
                    ps_z[:], ones_f[:], wexp_state[:], start=True, stop=True
                )

            # ---- AllReduce of (s_part, Z_part) ----
            sz = small.tile([B, UK + NZ], f32, tag="sz")
            if skip_s:
                nc.vector.memset(sz[:, :UK], 1.0)
            else:
                nc.vector.tensor_copy(out=sz[:, :UK], in_=ps_s[:])
            if t > 0:
                nc.vector.tensor_copy(out=sz[:, UK:], in_=ps_z[:])
            else:
                nc.vector.memset(sz[:, UK:], 0.0)
            cc_in = dram.tile([B, UK + NZ], f32, tag="cc_in")
            cc_out = dram.tile([B, UK + NZ], f32, tag="cc_out")
            nc.sync.dma_start(cc_in[:], sz[:])
            if fake_cc:
                nc.sync.dma_start(cc_out[:], cc_in[:])
            else:
                nc.gpsimd.collective_compute(
                    "AllReduce",
                    ADD,
                    replica_groups=[list(range(N_CORES))],
                    ins=[cc_in.opt()],
                    outs=[cc_out.opt()],
                )
            # replicate the 32-row result to all 128 partitions (4 groups)
            sz_all = small.tile([128, UK + NZ], f32, tag="sz_all")
            for g in range(4):
                nc.sync.dma_start(sz_all[32 * g : 32 * (g + 1), :], cc_out[:])

            # ---- softmax-normalize s, squash into v (on all 128 partitions) ----
            s_n = small.tile([128, UK], f32, tag="s_n")
            if t == 0:
                nc.scalar.mul(s_n[:], sz_all[:, :UK], 1.0 / C)
            else:
                zf = small.tile([128, U], f32, tag="zf")
                nc.vector.tensor_reduce(
                    out=zf[:],
                    in_=sz_all[:, UK:].rearrange("p (n u) -> p u n", n=NT),
                    axis=mybir.AxisListType.X,
                    op=ADD,
                )
                rz = small.tile([128, U], f32, tag="rz")
                nc.vector.reciprocal(rz[:], zf[:])
                nc.vector.tensor_tensor(
                    out=s_n[:].rearrange("b (u k) -> b u k", u=U),
                    in0=sz_all[:, :UK].rearrange("b (u k) -> b u k", u=U),
                    in1=rz[:].broadcast_to([128, U, K]),
                    op=MUL,
                )
            sq = small.tile([128, UK], f32, tag="sq")
            nc.scalar.square(sq[:], s_n[:])
            mag_sq = small.tile([128, K], f32, tag="mag_sq")
            nc.vector.tensor_reduce(
                out=mag_sq[:],
                in_=sq[:].rearrange("b (u k) -> b k u", u=U),
                axis=mybir.AxisListType.X,
                op=ADD,
            )
            mag = small.tile([128, K], f32, tag="mag")
            nc.scalar.sqrt(mag[:], mag_sq[:])
            den = small.tile([128, K], f32, tag="den")
            nc.vector.tensor_scalar_add(out=den[:], in0=mag_sq[:], scalar1=1.0)
            rden = small.tile([128, K], f32, tag="rden")
            nc.vector.reciprocal(rden[:], den[:])
            # fac = mag_sq / ((1 + mag_sq) * mag) = mag / (1 + mag_sq)
            fac = small.tile([128, K], f32, tag="fac")
            nc.vector.tensor_tensor(out=fac[:], in0=mag[:], in1=rden[:], op=MUL)
            if t < NUM_ITERS - 1:
                # write v/B into the 4 diagonal blocks of vblk
                facb = small.tile([128, K], f32, tag="facb")
                nc.vector.tensor_scalar_mul(out=facb[:], in0=fac[:], scalar1=1.0 / B)
                for g in range(4):
                    rows = slice(32 * g, 32 * (g + 1))
                    nc.vector.tensor_tensor(
                        out=vblk[rows, bass.ts(g, UK)].rearrange(
                            "b (u k) -> b k u", u=U
                        ),
                        in0=s_n[rows, :].rearrange("b (u k) -> b k u", u=U),
                        in1=facb[rows, :].broadcast_to([32, K, U]),
                        op=MUL,
                    )
            else:
                v_t = small.tile([B, UK], f32, tag="v_t")
                nc.vector.tensor_tensor(
                    out=v_t[:].rearrange("b (u k) -> b k u", u=U),
                    in0=s_n[:B, :].rearrange("b (u k) -> b k u", u=U),
                    in1=fac[:B, :].broadcast_to([B, K, U]),
                    op=MUL,
                )
                nc.sync.dma_start(v_out[:], v_t[:])


def _build():
    if "nc" in _CACHE:
        return _CACHE["nc"]
    nc = bacc.Bacc(
        "TRN2", target_bir_lowering=False, debug=False, num_devices=N_CORES
    )
    w_in = nc.dram_tensor("w", [C_LOC, U * K * I], f32r, kind="ExternalInput").ap()
    xn_in = nc.dram_tensor("xn", [128, 4 * C_LOC], f32r, kind="ExternalInput").ap()
    xt_in = nc.dram_tensor("xt", [128, NT * I * B], f32r, kind="ExternalInput").ap()
    v_out = nc.dram_tensor("v_out", [B, UK], f32, kind="ExternalOutput").ap()
    _body(nc, w_in, xn_in, xt_in, v_out)
    nc.compile()
    _CACHE["nc"] = nc
    return nc


def _prep_inputs(x, W):
    """Shard FULL inputs into the per-core DMA-friendly layouts."""
    x = np.asarray(x, dtype=np.float32)
    W = np.asarray(W, dtype=np.float32)
    in_maps = []
    for r in range(N_CORES):
        w_r = np.ascontiguousarray(W[r * C_LOC : (r + 1) * C_LOC]).reshape(C_LOC, -1)
        xs = x[:, :, r * C_LOC : (r + 1) * C_LOC]  # [B, I, C_LOC] view
        # xn[32*(i%4) + b, (i//4)*C_LOC + c] = xs[b, i, c]
        xn_r = np.ascontiguousarray(
            xs.transpose(1, 0, 2).reshape(4, 4, B, C_LOC).transpose(1, 2, 0, 3)
        ).reshape(128, 4 * C_LOC)
        # xt[cc, (tile, i, b)] = xs[b, i, tile*128 + cc]
        xt_r = np.ascontiguousarray(
            xs.reshape(B, I, NT, 128).transpose(3, 2, 1, 0)
        ).reshape(128, NT * I * B)
        in_maps.append({"w": w_r, "xn": xn_r, "xt": xt_r})
    return in_maps


def kernel(x, W):
    nc = _build()
    in_maps = _prep_inputs(x, W)
    res = run_bass_kernel_spmd(nc, in_maps, core_ids=list(range(N_CORES)))
    v = res.results[0]["v_out"]
    return v.reshape(B, U, K, 1).astype(np.float32)


def make_runner(nc, in_maps):
    """Device-resident repeat runner (timing infrastructure for test.py).

    Mirrors bass2jax.run_bass_via_pjrt's multi-core branch but keeps the
    jitted callable and device-resident inputs so executions can be queued
    asynchronously and timed without per-call host transfers.
    """
    import jax
    from concourse import bass2jax
    from concourse.bass2jax import _bass_exec_p, install_neuronx_cc_hook
    from jax.experimental.shard_map import shard_map
    from jax.sharding import Mesh, PartitionSpec, NamedSharding

    install_neuronx_cc_hook()
    n_cores = len(in_maps)
    partition_name = nc.partition_id_tensor.name if nc.partition_id_tensor else None
    in_names, out_names, out_avals, zero_outs = [], [], [], []
    for alloc in nc.m.functions[0].allocations:
        if not isinstance(alloc, mybir.MemoryLocationSet):
            continue
        name = alloc.memorylocations[0].name
        if alloc.kind == "ExternalInput":
            if name != partition_name:
                in_names.append(name)
        elif alloc.kind == "ExternalOutput":
            out_names.append(name)
            shape = tuple(alloc.tensor_shape)
            dtype = mybir.dt.np(alloc.dtype)
            out_avals.append(jax.core.ShapedArray(shape, dtype))
            zero_outs.append(np.zeros(shape, dtype))
    n_params = len(in_names)
    n_outs = len(out_avals)
    all_in_names = list(in_names) + out_names
    if partition_name is not None:
        all_in_names.append(partition_name)

    def _body(*args):
        operands = list(args)
        if partition_name is not None:
            operands.append(bass2jax.partition_id_tensor())
        outs = _bass_exec_p.bind(
            *operands,
            out_avals=tuple(out_avals),
            in_names=tuple(all_in_names),
            out_names=tuple(out_names),
            lowering_input_output_aliases=(),
            sim_require_finite=True,
            sim_require_nnan=True,
            nc=nc,
        )
        return tuple(outs)

    devices = jax.devices()[:n_cores]
    mesh = Mesh(np.asarray(devices), ("core",))
    in_specs = (PartitionSpec("core"),) * (n_params + n_outs)
    out_specs = (PartitionSpec("core"),) * len(out_names)
    donate = tuple(range(n_params, n_params + n_outs))
    sharded = jax.jit(
        shard_map(
            _body, mesh=mesh, in_specs=in_specs, out_specs=out_specs, check_rep=False
        ),
        donate_argnums=donate,
        keep_unused=True,
    )
    sh = NamedSharding(mesh, PartitionSpec("core"))
    concat_in = [
        jax.device_put(
            np.concatenate([np.asarray(in_maps[c][nm]) for c in range(n_cores)], 0),
            sh,
        )
        for nm in in_names
    ]
    for a in concat_in:
        a.block_until_ready()

    def run(n_iter=1):
        outs = None
        for _ in range(n_iter):
            zeros = [
                np.zeros((n_cores * z.shape[0], *z.shape[1:]), z.dtype)
                for z in zero_outs
            ]
            outs = sharded(*concat_in, *zeros)
        for o in outs:
            o.block_until_ready()
        return outs

    return run


# revision 20
# speedup vs baseline: 10672.8390x; 1.3949x over previous
"""DigitCaps dynamic-routing kernel for 8 Trainium2 NeuronCores.

Math (reference):
    u_hat[b,c,u,k] = sum_i W[c,u,k,i] * x[b,i,c]          (B=32, I=16, C=8192, U=32, K=16)
    b_ij = 0
    repeat 3x:
        c_ij  = softmax(b_ij, axis=c)
        s     = sum_c c_ij[c,u] * u_hat[b,c,u,k]
        v     = squash(s)    (norm over u, per (b,k))
        b_ij += mean_b <u_hat, v>
    return v

Strategy: shard C across the 8 cores (C_LOC = 1024 each).  u_hat (537 MB) is
never materialized; instead W is streamed from HBM once per routing iteration
(3 passes, 33.5 MB/core/pass) and each pass fuses the previous iteration's
agreement update a_{t-1} with the current weighted sum s_t:

  per 128-channel tile (pass t >= 1):
    VX_i[c,uk] = sum_b x[b,i,c] * (v_{t-1}[b,uk]/B)   (16 f32r matmuls via a
                 block-diagonal moving operand, grouped into double-buffered
                 multi-bank PSUM tiles so the DVE consumer pipelines)
    a[c,u]     = sum_{i,k} VX_i[c,(u,k)] * W[c,(u,k,i)]   (DVE mul + reduce)
    b_state   += a ; wexp = exp(b_state)                  (softmax numerator)
    W         *= wexp[c,u]  (in-place, DVE)
    s_part    += sum_{c,i} xT_i[c,b] * (wexp*W)[c,(u,k)]  (16 f32r matmuls, PE)
  Z = sum_c wexp is one ones-matmul over the stashed wexp state per pass.
  Then one ~100 KB AllReduce of (s_part, Z_part); the squash is computed
  redundantly on every core.  Softmax max-subtraction is skipped: b_ij stays
  within [-0.6, 0.6] for this problem so exp() cannot overflow.
  Dependent-instruction latency on this part (~2-4 us per edge) is the main
  non-DMA cost, so the structure favors few, large ops and deep pipelining.

Matmuls run in float32r (TF32-like, full PE rate); everything else is fp32.
"""

import contextlib

import numpy as np
import concourse.bass as bass
import concourse.bacc as bacc
import concourse.tile as tile
import concourse.mybir as mybir
from concourse.bass_utils import run_bass_kernel_spmd

B, I, C, U, K = 32, 16, 8192, 32, 16
UK = U * K
KI = K * I
N_CORES = 8
C_LOC = C // N_CORES
NT = C_LOC // 128
NUM_ITERS = 3

f32 = mybir.dt.float32
f32r = mybir.dt.float32r
MUL = mybir.AluOpType.mult
ADD = mybir.AluOpType.add
Exp = mybir.ActivationFunctionType.Exp
bf16 = mybir.dt.bfloat16

_CACHE = {}


def _body(nc, w_in, xn_in, xt_in, v_out, fake_cc=False, repeat=1, skip_a=False, skip_wd=False, skip_s=False, ig=2, pvbufs=2, prod_bf16=True, prodbufs=2):
    IG = ig  # i's per VX matmul group (pv2 spans IG PSUM banks)
    NG = I // IG
    tc_pools = [
        ("wpool", dict(bufs=3)),
        ("xpool", dict(bufs=1)),
        ("spool", dict(bufs=1)),
        ("prodpool", dict(bufs=prodbufs)),
        ("small", dict(bufs=1)),
        ("pvx", dict(bufs=pvbufs, space="PSUM")),
        ("pacc", dict(bufs=1, space="PSUM")),
        ("dram", dict(bufs=1, space="DRAM")),
    ]
    with tile.TileContext(nc) as tc, contextlib.ExitStack() as stack:
        pools = [stack.enter_context(tc.tile_pool(name=n, **kw)) for n, kw in tc_pools]
        wpool, xpool, spool, prodpool, small, pvx, pacc, dram = pools

        # ---- persistent tiles ----
        # xn: [(i%4, b) = 128 partitions, (i//4, c) free]
        xn = xpool.tile([128, 4 * C_LOC], f32r)
        nc.sync.dma_start(xn[:], xn_in[:])
        xt = xpool.tile([128, NT * I * B], f32r)
        nc.sync.dma_start(xt[:], xt_in[:])
        ones_f = xpool.tile([128, B], f32)
        nc.vector.memset(ones_f[:], 1.0)
        b_state = spool.tile([128, NT * U], f32)
        nc.vector.memset(b_state[:], 0.0)
        wexp_state = spool.tile([128, NT * U], f32)
        # vblk: block-diagonal moving operand for the VX matmuls
        # vblk[32*g + b, g*UK + z] = v[b, z] (1/B is folded into xn host-side)
        vblk = spool.tile([128, 4 * UK], f32r)
        nc.vector.memset(vblk[:].bitcast(f32), 0.0)

        xn3 = xn[:].rearrange("p (il c) -> p il c", il=4)
        xt4 = xt[:].rearrange("c (n i b) -> c n i b", n=NT, i=I)

        for rep in range(repeat):
          if rep > 0:
            nc.vector.memset(b_state[:], 0.0)
          for t in range(NUM_ITERS):
            ps_s = pacc.tile([B, UK], f32, tag="ps_s")
            for n in range(NT):
                wt = wpool.tile([128, U * K * I], f32r, tag="w")
                nc.sync.dma_start(wt[:], w_in[bass.ts(n, 128), :])
                # per-i view of W: [c, i, u, k] (walk u stride 256, k stride 16)
                w4 = wt[:].rearrange("c (u k i) -> c i u k", u=U, k=K)
                if t > 0 and not skip_a:
                    prod = prodpool.tile([128, U * I * K], bf16 if prod_bf16 else f32, tag="prod")
                    # prod layout (u, i, k): reduce over (i,k) contiguous per u
                    prod4 = prod[:].rearrange("c (u i k) -> c i u k", u=U, i=I)
                    for g in range(NG):
                        pv2 = pvx.tile([128, IG * UK], f32, tag="pv2")
                        for j in range(IG):
                            # i = g*IG + j ; full-128 contraction, zero rows
                            # of vblk outside group i%4 contribute nothing
                            i = g * IG + j
                            nc.tensor.matmul(
                                pv2[:, bass.ts(j, UK)],
                                xn3[:, i // 4, bass.ts(n, 128)],  # [128, 128]
                                vblk[:, bass.ts(i % 4, UK)],  # [128, 512]
                                start=True,
                                stop=True,
                            )
                        # prod[c, (u, g*IG..g*IG+IG, k)] = pv2 * W
                        nc.vector.tensor_tensor(
                            out=prod4[:, IG * g : IG * (g + 1)],
                            in0=pv2[:].rearrange("c (i z) -> c i z", i=IG).rearrange(
                                "c i (u k) -> c i u k", u=U
                            ),
                            in1=w4[:, IG * g : IG * (g + 1)],
                            op=MUL,
                        )
                    a_red = small.tile([128, U], f32, tag="a_red")
                    nc.vector.tensor_reduce(
                        out=a_red[:],
                        in_=prod[:].rearrange("c (u r) -> c u r", u=U),
                        axis=mybir.AxisListType.X,
                        op=ADD,
                    )
                    b_slice = b_state[:, bass.ts(n, U)]
                    nc.vector.tensor_tensor(
                        out=b_slice, in0=b_slice, in1=a_red[:], op=ADD
                    )
                    wexp = wexp_state[:, bass.ts(n, U)]
                    nc.scalar.activation(wexp, b_slice, Exp)
                    if not skip_wd:
                        # W *= wexp (in place) -> weighted W for the s-matmuls
                        w_u_r = wt[:].rearrange("c (u r) -> c u r", u=U)
                        nc.vector.tensor_tensor(
                            out=w_u_r,
                            in0=w_u_r,
                            in1=wexp.broadcast_to([128, U, KI]),
                            op=MUL,
                        )
                if t > 0 and skip_a:
                    b_slice = b_state[:, bass.ts(n, U)]
                    wexp = wexp_state[:, bass.ts(n, U)]
                    nc.scalar.activation(wexp, b_slice, Exp)
                if skip_s:
                    continue
                for i in range(I):
                    nc.tensor.matmul(
                        ps_s[:],
                        xt4[:, n, i, :],  # [128c, 32b]
                        w4[:, i],  # [128c, U, K]
                        start=(n == 0 and i == 0),
                        stop=(n == NT - 1 and i == I - 1),
                    )

            # ---- Z = sum_c wexp (one matmul over the whole pass state) ----
            NZ = U
            if t > 0:
                ps_z = pacc.tile([B, NT * U], f32, tag="ps_z")
                nc.tensor.matmul(
                    ps_z[:], ones_f[:], wexp_state[:], start=True, stop=True
                )

            # ---- AllReduce of (s_part, Z_part) ----
            sz = small.tile([B, UK + NZ], f32, tag="sz")
            if skip_s:
                nc.vector.memset(sz[:, :UK], 1.0)
            else:
                nc.vector.tensor_copy(out=sz[:, :UK], in_=ps_s[:])
            if t > 0:
                # fold the per-tile partial Z sums while copying out of PSUM
                nc.vector.tensor_reduce(
                    out=sz[:, UK:],
                    in_=ps_z[:].rearrange("b (n u) -> b u n", n=NT),
                    axis=mybir.AxisListType.X,
                    op=ADD,
                )
            else:
                nc.vector.memset(sz[:, UK:], 0.0)
            cc_in = dram.tile([B, UK + NZ], f32, tag="cc_in")
            cc_out = dram.tile([B, UK + NZ], f32, tag="cc_out")
            nc.sync.dma_start(cc_in[:], sz[:])
            if fake_cc:
                nc.sync.dma_start(cc_out[:], cc_in[:])
            else:
                nc.gpsimd.collective_compute(
                    "AllReduce",
                    ADD,
                    replica_groups=[list(range(N_CORES))],
                    ins=[cc_in.opt()],
                    outs=[cc_out.opt()],
                )
            # replicate the 32-row result to all 128 partitions (4 groups)
            sz_all = small.tile([128, UK + NZ], f32, tag="sz_all")
            for g in range(4):
                nc.sync.dma_start(sz_all[32 * g : 32 * (g + 1), :], cc_out[:])

            # ---- softmax-normalize s, squash into v (on all 128 partitions) ----
            s_n = small.tile([128, UK], f32, tag="s_n")
            if t == 0:
                nc.scalar.mul(s_n[:], sz_all[:, :UK], 1.0 / C)
            else:
                rz = small.tile([128, U], f32, tag="rz")
                nc.vector.reciprocal(rz[:], sz_all[:, UK:])
                nc.vector.tensor_tensor(
                    out=s_n[:].rearrange("b (u k) -> b u k", u=U),
                    in0=sz_all[:, :UK].rearrange("b (u k) -> b u k", u=U),
                    in1=rz[:].broadcast_to([128, U, K]),
                    op=MUL,
                )
            sq = small.tile([128, UK], f32, tag="sq")
            nc.vector.tensor_tensor(out=sq[:], in0=s_n[:], in1=s_n[:], op=MUL)
            mag_sq = small.tile([128, K], f32, tag="mag_sq")
            nc.vector.tensor_reduce(
                out=mag_sq[:],
                in_=sq[:].rearrange("b (u k) -> b k u", u=U),
                axis=mybir.AxisListType.X,
                op=ADD,
            )
            mag = small.tile([128, K], f32, tag="mag")
            nc.scalar.sqrt(mag[:], mag_sq[:])
            den = small.tile([128, K], f32, tag="den")
            nc.vector.tensor_scalar_add(out=den[:], in0=mag_sq[:], scalar1=1.0)
            rden = small.tile([128, K], f32, tag="rden")
            nc.vector.reciprocal(rden[:], den[:])
            # fac = mag_sq / ((1 + mag_sq) * mag) = mag / (1 + mag_sq)
            fac = small.tile([128, K], f32, tag="fac")
            nc.vector.tensor_tensor(out=fac[:], in0=mag[:], in1=rden[:], op=MUL)
            if t < NUM_ITERS - 1:
                # write v into the 4 diagonal blocks of vblk
                for g in range(4):
                    rows = slice(32 * g, 32 * (g + 1))
                    nc.vector.tensor_tensor(
                        out=vblk[rows, bass.ts(g, UK)].rearrange(
                            "b (u k) -> b k u", u=U
                        ),
                        in0=s_n[rows, :].rearrange("b (u k) -> b k u", u=U),
                        in1=fac[rows, :].broadcast_to([32, K, U]),
                        op=MUL,
                    )
            else:
                v_t = small.tile([B, UK], f32, tag="v_t")
                nc.vector.tensor_tensor(
                    out=v_t[:].rearrange("b (u k) -> b k u", u=U),
                    in0=s_n[:B, :].rearrange("b (u k) -> b k u", u=U),
                    in1=fac[:B, :].broadcast_to([B, K, U]),
                    op=MUL,
                )
                nc.sync.dma_start(v_out[:], v_t[:])


def _build():
    if "nc" in _CACHE:
        return _CACHE["nc"]
    nc = bacc.Bacc(
        "TRN2", target_bir_lowering=False, debug=False, num_devices=N_CORES
    )
    w_in = nc.dram_tensor("w", [C_LOC, U * K * I], f32r, kind="ExternalInput").ap()
    xn_in = nc.dram_tensor("xn", [128, 4 * C_LOC], f32r, kind="ExternalInput").ap()
    xt_in = nc.dram_tensor("xt", [128, NT * I * B], f32r, kind="ExternalInput").ap()
    v_out = nc.dram_tensor("v_out", [B, UK], f32, kind="ExternalOutput").ap()
    _body(nc, w_in, xn_in, xt_in, v_out)
    nc.compile()
    _CACHE["nc"] = nc
    return nc


def _prep_inputs(x, W):
    """Shard FULL inputs into the per-core DMA-friendly layouts."""
    x = np.asarray(x, dtype=np.float32)
    W = np.asarray(W, dtype=np.float32)
    in_maps = []
    for r in range(N_CORES):
        w_r = np.ascontiguousarray(W[r * C_LOC : (r + 1) * C_LOC]).reshape(C_LOC, -1)
        xs = x[:, :, r * C_LOC : (r + 1) * C_LOC]  # [B, I, C_LOC] view
        # xn[32*(i%4) + b, (i//4)*C_LOC + c] = xs[b, i, c]
        xn_r = np.ascontiguousarray(
            xs.transpose(1, 0, 2).reshape(4, 4, B, C_LOC).transpose(1, 2, 0, 3)
        ).reshape(128, 4 * C_LOC) * np.float32(1.0 / B)
        # xt[cc, (tile, i, b)] = xs[b, i, tile*128 + cc]
        xt_r = np.ascontiguousarray(
            xs.reshape(B, I, NT, 128).transpose(3, 2, 1, 0)
        ).reshape(128, NT * I * B)
        in_maps.append({"w": w_r, "xn": xn_r, "xt": xt_r})
    return in_maps


def kernel(x, W):
    nc = _build()
    in_maps = _prep_inputs(x, W)
    res = run_bass_kernel_spmd(nc, in_maps, core_ids=list(range(N_CORES)))
    v = res.results[0]["v_out"]
    return v.reshape(B, U, K, 1).astype(np.float32)


def make_runner(nc, in_maps):
    """Device-resident repeat runner (timing infrastructure for test.py).

    Mirrors bass2jax.run_bass_via_pjrt's multi-core branch but keeps the
    jitted callable and device-resident inputs so executions can be queued
    asynchronously and timed without per-call host transfers.
    """
    import jax
    from concourse import bass2jax
    from concourse.bass2jax import _bass_exec_p, install_neuronx_cc_hook
    from jax.experimental.shard_map import shard_map
    from jax.sharding import Mesh, PartitionSpec, NamedSharding

    install_neuronx_cc_hook()
    n_cores = len(in_maps)
    partition_name = nc.partition_id_tensor.name if nc.partition_id_tensor else None
    in_names, out_names, out_avals, zero_outs = [], [], [], []
    for alloc in nc.m.functions[0].allocations:
        if not isinstance(alloc, mybir.MemoryLocationSet):
            continue
        name = alloc.memorylocations[0].name
        if alloc.kind == "ExternalInput":
            if name != partition_name:
                in_names.append(name)
        elif alloc.kind == "ExternalOutput":
            out_names.append(name)
            shape = tuple(alloc.tensor_shape)
            dtype = mybir.dt.np(alloc.dtype)
            out_avals.append(jax.core.ShapedArray(shape, dtype))
            zero_outs.append(np.zeros(shape, dtype))
    n_params = len(in_names)
    n_outs = len(out_avals)
    all_in_names = list(in_names) + out_names
    if partition_name is not None:
        all_in_names.append(partition_name)

    def _body(*args):
        operands = list(args)
        if partition_name is not None:
            operands.append(bass2jax.partition_id_tensor())
        outs = _bass_exec_p.bind(
            *operands,
            out_avals=tuple(out_avals),
            in_names=tuple(all_in_names),
            out_names=tuple(out_names),
            lowering_input_output_aliases=(),
            sim_require_finite=True,
            sim_require_nnan=True,
            nc=nc,
        )
        return tuple(outs)

    devices = jax.devices()[:n_cores]
    mesh = Mesh(np.asarray(devices), ("core",))
    in_specs = (PartitionSpec("core"),) * (n_params + n_outs)
    out_specs = (PartitionSpec("core"),) * len(out_names)
    donate = tuple(range(n_params, n_params + n_outs))
    sharded = jax.jit(
        shard_map(
            _body, mesh=mesh, in_specs=in_specs, out_specs=out_specs, check_rep=False
        ),
        donate_argnums=donate,
        keep_unused=True,
    )
    sh = NamedSharding(mesh, PartitionSpec("core"))
    concat_in = [
        jax.device_put(
            np.concatenate([np.asarray(in_maps[c][nm]) for c in range(n_cores)], 0),
            sh,
        )
        for nm in in_names
    ]
    for a in concat_in:
        a.block_until_ready()

    def run(n_iter=1):
        outs = None
        for _ in range(n_iter):
            zeros = [
                np.zeros((n_cores * z.shape[0], *z.shape[1:]), z.dtype)
                for z in zero_outs
            ]
            outs = sharded(*concat_in, *zeros)
        for o in outs:
            o.block_until_ready()
        return outs

    return run
